# revision 1
# baseline (speedup 1.0000x reference)
"""Multi-head causal attention with RoPE on 8 Trainium2 NeuronCores.

Sharding: tensor-parallel over heads x data-parallel over batch.
Core c handles batch b = c//4 and heads [4*(c%4), 4*(c%4)+4) (Hl=256 of Hd=1024).
Each core computes q/k/v projections for its head slice (column-split Wq/Wk/Wv),
RoPE, causal softmax attention, and a partial output projection (row-split Wo).
The host sums the 4 partial outputs per batch (the "all-reduce").

Device layouts (per core, S=2048, E=1024, Hl=256, D=64):
  xT   [E, S]    x transposed (host-side) so E rides the partition dim
  qT/kT slabs [128, S] x2: partitions = 2 heads x 64 dims, free = seq
  v    16 tiles [128, 260]: partitions = seq chunk, free = 4 heads x (64 dims + ones col)
  scores computed transposed (keys on partitions), softmax Z via ones-column of v,
  normalization by 1/Z broadcast via a DRAM-roundtrip partition-broadcast DMA.

All matmul operands are fp16 (1 cyc/row with overlapped fast weight loads;
fp32 PSUM accumulation; ~5e-4 end-to-end rel err measured on HW). The two
heads of a slab run in lockstep on disjoint PE row groups, the attention is
software-pipelined over (slab, block, chunk-pair) units so ACT never starves
at block boundaries, and the mostly-masked trailing diagonal chunk-pair of
each block is width-trimmed in the score/exp/pv path.
"""
import sys

sys.path.insert(0, "/opt/trn_rl_repo")
import numpy as np  # noqa: E402

N_HEADS = 16
B, S, E, HD = 2, 2048, 1024, 1024
D = HD // N_HEADS  # 64
HPC = 4            # heads per core
HL = HPC * D       # 256
NCORES = 8
ROPE_BASE = 10000.0

_built = None


def _build_nc():
    import concourse.bass as bass
    import concourse.tile as tile
    from concourse import bacc, mybir

    F32 = mybir.dt.float32
    F32R = mybir.dt.float32r
    F16 = mybir.dt.float16
    Exp = mybir.ActivationFunctionType.Exp
    is_ge = mybir.AluOpType.is_ge
    ts = bass.ts

    nc = bacc.Bacc("TRN2", target_bir_lowering=False, debug=False)
    xT_d = nc.dram_tensor("xT", [E, S], F16, kind="ExternalInput").ap()
    wq_d = nc.dram_tensor("wq", [E, HL], F16, kind="ExternalInput").ap()
    wk_d = nc.dram_tensor("wk", [E, HL], F16, kind="ExternalInput").ap()
    wv_d = nc.dram_tensor("wv", [E, HL], F16, kind="ExternalInput").ap()
    wo_d = nc.dram_tensor("wo", [HL, E], F16, kind="ExternalInput").ap()
    cos_d = nc.dram_tensor("cosx", [128, S], F32, kind="ExternalInput").ap()
    sin_d = nc.dram_tensor("sinx", [128, S], F32, kind="ExternalInput").ap()
    out_d = nc.dram_tensor("out", [S, E], F32, kind="ExternalOutput").ap()
    zscr_d = nc.dram_tensor("zscr", [HPC, S], F32).ap()  # internal scratch

    ECH = E // 128   # 8 e-chunks
    SCH = S // 128   # 16 seq chunks
    SB = S // 512    # 4 seq blocks
    swap_mask = []
    for i in range(16):
        swap_mask += [2 * i + 1, 2 * i]

    with tile.TileContext(nc) as tc:
        with (
            tc.tile_pool(name="persist", bufs=1) as pp,
            tc.tile_pool(name="evict", bufs=6) as ev,
        ):
            # persistent tiles
            qT = [pp.tile([128, S], F16, tag=f"qT{c}", name=f"qT{c}") for c in range(2)]
            kT = [pp.tile([128, S], F16, tag=f"kT{c}", name=f"kT{c}") for c in range(2)]
            vt = [pp.tile([128, HPC * (D + 1)], F16, tag=f"v{t}", name=f"v{t}")
                  for t in range(SCH)]
            oT = [pp.tile([128, S], F16, tag=f"oT{c}", name=f"oT{c}") for c in range(2)]
            cosx = pp.tile([128, S], F32R, tag="cosx", name="cosx")
            sinx = pp.tile([128, S], F32R, tag="sinx", name="sinx")
            wo_t = pp.tile([128, 2, E], F16, tag="wo", name="wo")

            # small/constant loads on the scalar queue, weights spread across
            # queues so nothing waits behind the big xT stream


            # ---------------- Phase B: projections + RoPE ----------------
            with (
                tc.tile_pool(name="bx", bufs=1) as bx,
                tc.tile_pool(name="bswp", bufs=2) as bswp,
                tc.tile_pool(name="bps", bufs=8, space="PSUM") as bps,
            ):
                wq_t = bx.tile([128, ECH, HL], F16, tag="wq", name="wq")
                wk_t = bx.tile([128, ECH, HL], F16, tag="wk", name="wk")
                wv_t = bx.tile([128, ECH, HL], F16, tag="wv", name="wv")
                def wdma(eng, w_t_, w_d_):
                    eng.dma_start(
                        out=w_t_[:],
                        in_=w_d_.rearrange("(c p) m -> p c m", p=128),
                    )
                xt = [bx.tile([128, S], F16, tag=f"x{e}", name=f"x{e}")
                      for e in range(ECH)]

                def xdma(eng, e, j):
                    eng.dma_start(
                        out=xt[e][:, ts(j, 512)],
                        in_=xT_d[e * 128:(e + 1) * 128, ts(j, 512)],
                    )
                # feed order: j0/j1 slices of every chunk first so early
                # projection psum groups complete within a few us
                wdma(nc.sync, wq_t, wq_d)
                wdma(nc.scalar, wk_t, wk_d)
                for j in range(SB):
                    for e in range(0, ECH, 2):
                        xdma(nc.sync, e, j)
                    for e in range(1, ECH, 2):
                        xdma(nc.scalar, e, j)
                nc.scalar.dma_start(out=cosx[:], in_=cos_d.bitcast(F32R))
                nc.scalar.dma_start(out=sinx[:], in_=sin_d.bitcast(F32R))
                wdma(nc.scalar, wv_t, wv_d)
                nc.scalar.dma_start(
                    out=wo_t[:],
                    in_=wo_d.rearrange("(c p) e -> p c e", p=128),
                )

                # q/k projections -> transposed slabs, RoPE interleaved so the
                # DVE rope work overlaps the next chunk's PE matmuls
                def rope(dest, c):
                    sw = bswp.tile([128, S], F16, tag="swp", name="swp")
                    nc.vector.stream_shuffle(
                        out=sw[:], in_=dest[c][:], mask=swap_mask
                    )
                    nc.vector.tensor_mul(out=sw[:], in0=sw[:], in1=sinx[:])
                    nc.vector.tensor_mul(out=dest[c][:], in0=dest[c][:], in1=cosx[:])
                    nc.vector.tensor_add(out=dest[c][:], in0=dest[c][:], in1=sw[:])

                for w_t_, dest in ((wq_t, qT), (wk_t, kT)):
                    for m in range(2):
                        for j in range(SB):
                            ps = bps.tile([128, 512], F32, tag="mm", name="mm")
                            for e in range(ECH):
                                nc.tensor.matmul(
                                    ps[:],
                                    w_t_[:, e, m * 128:(m + 1) * 128],
                                    xt[e][:, ts(j, 512)],
                                    start=(e == 0),
                                    stop=(e == ECH - 1),
                                )
                            nc.vector.tensor_copy(
                                out=dest[m][:, ts(j, 512)], in_=ps[:]
                            )
                        if m == 0:
                            rope(dest, m)
                # v projection -> seq-partition tiles with ones column
                for t in range(SCH):
                    nc.gpsimd.memset(
                        vt[t].rearrange("p (h c) -> p h c", c=D + 1)[:, :, D:D + 1],
                        1.0,
                    )
                    ps = bps.tile([128, HL], F32, tag="mm", name="mmv")
                    for e in range(ECH):
                        nc.tensor.matmul(
                            ps[:],
                            xt[e][:, ts(t, 128)],
                            wv_t[:, e, :],
                            start=(e == 0),
                            stop=(e == ECH - 1),
                        )
                    nc.vector.tensor_copy(
                        out=vt[t].rearrange("p (h c) -> p h c", c=D + 1)[:, :, 0:D],
                        in_=ps.rearrange("p (h c) -> p h c", c=D),
                    )
                rope(qT, 1)
                rope(kT, 1)

            # ---------------- Phase C: attention, two heads in lockstep -------
            with (
                tc.tile_pool(name="cexp", bufs=6) as cexp,
                tc.tile_pool(name="cz", bufs=4) as cz,
                tc.tile_pool(name="crb", bufs=3) as crb,
                tc.tile_pool(name="csc", bufs=3, space="PSUM") as csc,
                tc.tile_pool(name="cpv", bufs=1, space="PSUM") as cpv,
            ):
                def qksv(c):
                    hs = [2 * c, 2 * c + 1]
                    qs = [qT[c][0:64, :], qT[c][64:128, :]]
                    ks = [kT[c][0:64, :], kT[c][64:128, :]]
                    vs = [
                        [vt[t].rearrange("p (h c) -> p h c", c=D + 1)[:, h, :]
                         for t in range(SCH)]
                        for h in hs
                    ]
                    return hs, qs, ks, vs

                # software-pipelined over (slab, block, chunk-pair) units: the
                # next unit's score matmuls are emitted before the previous
                # unit's exp/pv consumption so ACT stays fed across block
                # boundaries
                units = []
                for c in range(2):
                    for j in range(SB):
                        nt = 4 * (j + 1)
                        for tp in range(nt // 2):
                            units.append((c, j, tp, nt))
                sc_of = {}
                pv_of = {}

                def emit_sc(u):
                    c, j, tp, nt = u
                    _, qs, ks, _ = qksv(c)
                    sc = [csc.tile([128, 1024], F32, tag="sc", name="sc")
                          for _ in range(2)]
                    # head 0 on PE rows 0-63, head 1 on rows 64-127: adjacent
                    # pairs overlap in the array
                    for half in range(2):
                        t = 2 * tp + half
                        # trim only the last pair (offs 256/384), whose exp is
                        # split to match; earlier pairs keep full width
                        off = max(t * 128 - j * 512, 0) if tp == nt // 2 - 1 else 0
                        for i in range(2):
                            nc.tensor.matmul(
                                sc[i][:, half * 512 + off:(half + 1) * 512],
                                ks[i][:, ts(t, 128)],
                                qs[i][:, j * 512 + off:(j + 1) * 512],
                                start=True,
                                stop=True,
                            )
                    sc_of[u] = sc

                def emit_consume(u):
                    c, j, tp, nt = u
                    hs, _, _, vs = qksv(c)
                    if tp == 0:
                        pv_of[(c, j)] = [
                            cpv.tile([65, 512], F32, tag=f"pv{i}", name=f"pv{i}")
                            for i in range(2)
                        ]
                    pv = pv_of[(c, j)]
                    sc = sc_of.pop(u)
                    trimmed = (tp == nt // 2 - 1)  # offs 256/384: mostly masked
                    exm = []
                    for i in range(2):
                        ex = cexp.tile([128, 1024], F16, tag="ex", name="ex")
                        if trimmed:
                            nc.scalar.activation(
                                out=ex[:, 256:512], in_=sc[i][:, 256:512],
                                func=Exp, scale=0.125,
                            )
                            nc.scalar.activation(
                                out=ex[:, 896:1024], in_=sc[i][:, 896:1024],
                                func=Exp, scale=0.125,
                            )
                        else:
                            nc.scalar.activation(
                                out=ex[:], in_=sc[i][:], func=Exp, scale=0.125
                            )
                        exm.append(ex)
                    for half in range(2):
                        t = 2 * tp + half
                        off = max(t * 128 - j * 512, 0) if trimmed else 0
                        if t >= nt - 4:  # diagonal chunk: causal mask
                            for i in range(2):
                                nc.gpsimd.affine_select(
                                    out=exm[i][:, half * 512 + off:
                                               (half + 1) * 512],
                                    in_=exm[i][:, half * 512 + off:
                                              (half + 1) * 512],
                                    compare_op=is_ge,
                                    fill=0.0,
                                    base=(j * 512 - t * 128) + off,
                                    channel_multiplier=-1,
                                    pattern=[[1, 512 - off]],
                                )
                        for i in range(2):
                            nc.tensor.matmul(
                                pv[i][:, off:512],
                                vs[i][t],
                                exm[i][:, half * 512 + off:(half + 1) * 512],
                                start=(t == 0),
                                stop=(t == nt - 1),
                            )
                    if tp == nt // 2 - 1:
                        # end of block: evict + per-block softmax normalization
                        # (Z -> DRAM -> partition-broadcast, invert, scale)
                        zq = cz.tile([65, 2, 512], F32, tag="zq", name="zq")
                        for i in range(2):
                            nc.vector.tensor_copy(
                                out=oT[c][i * 64:(i + 1) * 64, ts(j, 512)],
                                in_=pv[i][0:64, :],
                            )
                            nc.vector.tensor_copy(
                                out=zq[64:65, i, :], in_=pv[i][64:65, :]
                            )
                        for i in range(2):
                            nc.sync.dma_start(
                                out=zscr_d[hs[i], ts(j, 512)],
                                in_=zq[64:65, i, :],
                            )
                        rb = crb.tile([128, 512], F32, tag="rb", name="rb")
                        for i in range(2):
                            nc.sync.dma_start(
                                out=rb[i * 64:(i + 1) * 64, :],
                                in_=zscr_d[hs[i]:hs[i] + 1, ts(j, 512)]
                                .to_broadcast((64, 512)),
                            )
                        rbr = crb.tile([128, 512], F32, tag="rbr", name="rbr")
                        nc.vector.reciprocal(out=rbr[:], in_=rb[:])
                        nc.vector.tensor_mul(
                            out=oT[c][:, ts(j, 512)],
                            in0=oT[c][:, ts(j, 512)],
                            in1=rbr[:],
                        )

                emit_sc(units[0])
                for un in range(1, len(units)):
                    emit_sc(units[un])
                    emit_consume(units[un - 1])
                emit_consume(units[-1])

            # ---------------- Phase D: output projection (row-split Wo) --------
            with tc.tile_pool(name="dps", bufs=8, space="PSUM") as dps:
                for t in range(SCH):
                    for n in range(2):
                        ps = dps.tile([128, 512], F32, tag="wo", name="wops")
                        for c in range(2):
                            nc.tensor.matmul(
                                ps[:],
                                oT[c][:, ts(t, 128)],
                                wo_t[:, c, ts(n, 512)],
                                start=(c == 0),
                                stop=(c == 1),
                            )
                        ot = ev.tile([128, 512], F32, tag="out", name="oev")
                        if (2 * t + n) % 2 == 0:
                            nc.vector.tensor_copy(out=ot[:], in_=ps[:])
                        else:
                            nc.scalar.copy(out=ot[:], in_=ps[:])
                        nc.sync.dma_start(
                            out=out_d[ts(t, 128), ts(n, 512)], in_=ot[:]
                        )

    nc.compile()
    return nc


def _rope_tables():
    iexp = np.arange(0, D, 2, dtype=np.float32) / np.float32(D)
    inv_freq = np.reciprocal(np.power(np.float32(ROPE_BASE), iexp))  # (32,) f32
    ang = np.arange(S, dtype=np.float32)[:, None] * inv_freq[None, :]  # (S, 32)
    cos = np.cos(ang).astype(np.float32)  # (S, 32)
    sin = np.sin(ang).astype(np.float32)
    cosx = np.empty((64, S), dtype=np.float32)
    sinx = np.empty((64, S), dtype=np.float32)
    cosx[0::2] = cos.T
    cosx[1::2] = cos.T
    sinx[0::2] = -sin.T
    sinx[1::2] = sin.T
    return np.tile(cosx, (2, 1)), np.tile(sinx, (2, 1))  # (128, S) each


def get_nc():
    global _built
    if _built is None:
        _built = _build_nc()
    return _built


def make_in_maps(x, Wq, Wk, Wv, Wo):
    cosx, sinx = _rope_tables()
    in_maps = []
    for c in range(NCORES):
        b, g = c // 4, c % 4
        sl = slice(g * HL, (g + 1) * HL)
        in_maps.append({
            "xT": np.ascontiguousarray(x[b].T).astype(np.float16),
            "wq": np.ascontiguousarray(Wq[:, sl]).astype(np.float16),
            "wk": np.ascontiguousarray(Wk[:, sl]).astype(np.float16),
            "wv": np.ascontiguousarray(Wv[:, sl]).astype(np.float16),
            "wo": np.ascontiguousarray(Wo[sl, :]).astype(np.float16),
            "cosx": cosx,
            "sinx": sinx,
        })
    return in_maps


def gather(results):
    out = np.empty((B, S, E), dtype=np.float32)
    for b in range(B):
        acc = results[4 * b]["out"].astype(np.float32).copy()
        for g in range(1, 4):
            acc += results[4 * b + g]["out"]
        out[b] = acc
    return out


def kernel(x, Wq, Wk, Wv, Wo):
    from concourse.bass_utils import run_bass_kernel_spmd

    nc = get_nc()
    in_maps = make_in_maps(
        np.asarray(x), np.asarray(Wq), np.asarray(Wk), np.asarray(Wv), np.asarray(Wo)
    )
    res = run_bass_kernel_spmd(nc, in_maps, list(range(NCORES)))
    return gather(res.results)



# revision 6
# speedup vs baseline: 1.0705x; 1.0705x over previous
"""Multi-head causal attention with RoPE on 8 Trainium2 NeuronCores.

Sharding: tensor-parallel over heads x data-parallel over batch.
Core c handles batch b = c//4 and heads [4*(c%4), 4*(c%4)+4) (Hl=256 of Hd=1024).
Each core computes q/k/v projections for its head slice (column-split Wq/Wk/Wv),
RoPE, causal softmax attention, and a partial output projection (row-split Wo).
The host sums the 4 partial outputs per batch (the "all-reduce").

Device layouts (per core, S=2048, E=1024, Hl=256, D=64):
  xT   [E, S]    x transposed (host-side) so E rides the partition dim
  qT/kT slabs [128, S] x2: partitions = 2 heads x 64 dims, free = seq
  v    16 tiles [128, 260]: partitions = seq chunk, free = 4 heads x (64 dims + ones col)
  scores computed transposed (keys on partitions), softmax Z via ones-column of v,
  normalization by 1/Z broadcast via a DRAM-roundtrip partition-broadcast DMA.

All matmul operands are fp16 (fp32 PSUM accumulation). Per-chunk score tiles
pack both heads of a slab side by side ([128, 2x512]) so each chunk needs one
exp ACTIVATE and one affine_select. Units are software-pipelined per chunk and
blocks interleave the two slabs so block-end normalization of one slab hides
under the other's matmuls. Warmup matmuls during the input-DMA ramp keep the
PE HAM clock-gate warm; inputs stream over four DMA queues; partial outputs
are written fp16 and summed on the host.
"""
import sys

sys.path.insert(0, "/opt/trn_rl_repo")
import numpy as np  # noqa: E402

N_HEADS = 16
B, S, E, HD = 2, 2048, 1024, 1024
D = HD // N_HEADS  # 64
HPC = 4            # heads per core
HL = HPC * D       # 256
NCORES = 8
ROPE_BASE = 10000.0

_built = None


def _build_nc():
    import concourse.bass as bass
    import concourse.tile as tile
    from concourse import bacc, mybir

    F32 = mybir.dt.float32
    F16 = mybir.dt.float16
    Exp = mybir.ActivationFunctionType.Exp
    is_ge = mybir.AluOpType.is_ge
    ts = bass.ts

    nc = bacc.Bacc("TRN2", target_bir_lowering=False, debug=False)
    xT_d = nc.dram_tensor("xT", [E, S], F16, kind="ExternalInput").ap()
    wq_d = nc.dram_tensor("wq", [E, HL], F16, kind="ExternalInput").ap()
    wk_d = nc.dram_tensor("wk", [E, HL], F16, kind="ExternalInput").ap()
    wv_d = nc.dram_tensor("wv", [E, HL], F16, kind="ExternalInput").ap()
    wo_d = nc.dram_tensor("wo", [HL, E], F16, kind="ExternalInput").ap()
    cos_d = nc.dram_tensor("cosx", [128, S], F16, kind="ExternalInput").ap()
    sin_d = nc.dram_tensor("sinx", [128, S], F16, kind="ExternalInput").ap()
    out_d = nc.dram_tensor("out", [S, E], F16, kind="ExternalOutput").ap()
    zscr_d = nc.dram_tensor("zscr", [HPC, S], F32).ap()  # internal scratch

    ECH = E // 128   # 8 e-chunks
    SCH = S // 128   # 16 seq chunks
    SB = S // 512    # 4 seq blocks
    swap_mask = []
    for i in range(16):
        swap_mask += [2 * i + 1, 2 * i]

    with tile.TileContext(nc) as tc:
        with (
            tc.tile_pool(name="persist", bufs=1) as pp,
            tc.tile_pool(name="evict", bufs=6) as ev,
        ):
            # persistent tiles
            qT = [pp.tile([128, S], F16, tag=f"qT{c}", name=f"qT{c}") for c in range(2)]
            kT = [pp.tile([128, S], F16, tag=f"kT{c}", name=f"kT{c}") for c in range(2)]
            vt = [pp.tile([128, HPC * (D + 1)], F16, tag=f"v{t}", name=f"v{t}")
                  for t in range(SCH)]
            oT = [pp.tile([128, S], F16, tag=f"oT{c}", name=f"oT{c}") for c in range(2)]
            cosx = pp.tile([128, S], F16, tag="cosx", name="cosx")
            sinx = pp.tile([128, S], F16, tag="sinx", name="sinx")
            wo_t = pp.tile([128, 2, E], F16, tag="wo", name="wo")

            # ---------------- Phase B: projections + RoPE ----------------
            with (
                tc.tile_pool(name="bx", bufs=1) as bx,
                tc.tile_pool(name="bswp", bufs=2) as bswp,
                tc.tile_pool(name="bps", bufs=7, space="PSUM") as bps,
                tc.tile_pool(name="bwarm", bufs=1, space="PSUM") as bwarm,
            ):
                # PE warmup during the input-DMA ramp: junk matmuls keep the
                # HAM activity monitor busy so real matmuls start at 2.4 GHz
                junk = bx.tile([128, 512], F16, tag="junk", name="junk")
                wps = bwarm.tile([128, 512], F32, tag="warm", name="warm")
                nc.gpsimd.memset(junk[:], 0.0)
                for _ in range(24):
                    nc.tensor.matmul(wps[:], junk[:, 0:128], junk[:],
                                     start=True, stop=True)

                wq_t = bx.tile([128, ECH, HL], F16, tag="wq", name="wq")
                wk_t = bx.tile([128, ECH, HL], F16, tag="wk", name="wk")
                wv_t = bx.tile([128, ECH, HL], F16, tag="wv", name="wv")
                def wdma(eng, w_t_, w_d_):
                    eng.dma_start(
                        out=w_t_[:],
                        in_=w_d_.rearrange("(c p) m -> p c m", p=128),
                    )
                xt = [bx.tile([128, S], F16, tag=f"x{e}", name=f"x{e}")
                      for e in range(ECH)]

                def xdma(eng, e, j):
                    eng.dma_start(
                        out=xt[e][:, ts(j, 512)],
                        in_=xT_d[e * 128:(e + 1) * 128, ts(j, 512)],
                    )
                # feed order: j0/j1 slices of every chunk first so early
                # projection psum groups complete within a few us; four
                # queues share the input stream
                wdma(nc.sync, wq_t, wq_d)
                wdma(nc.scalar, wk_t, wk_d)
                for j in range(SB):
                    for e in range(0, ECH, 2):
                        xdma(nc.sync, e, j)
                    for e in range(1, ECH, 2):
                        xdma(nc.gpsimd, e, j)
                nc.scalar.dma_start(out=cosx[:], in_=cos_d)
                nc.scalar.dma_start(out=sinx[:], in_=sin_d)
                wdma(nc.scalar, wv_t, wv_d)
                nc.scalar.dma_start(
                    out=wo_t[:],
                    in_=wo_d.rearrange("(c p) e -> p c e", p=128),
                )

                # q/k projections -> transposed slabs, RoPE interleaved so the
                # DVE rope work overlaps the next chunk's PE matmuls
                def rope(dest, c):
                    sw = bswp.tile([128, S], F16, tag="swp", name="swp")
                    nc.vector.stream_shuffle(
                        out=sw[:], in_=dest[c][:], mask=swap_mask
                    )
                    nc.vector.tensor_mul(out=sw[:], in0=sw[:], in1=sinx[:])
                    nc.vector.tensor_mul(out=dest[c][:], in0=dest[c][:], in1=cosx[:])
                    nc.vector.tensor_add(out=dest[c][:], in0=dest[c][:], in1=sw[:])

                nev = 0
                for w_t_, dest in ((wq_t, qT), (wk_t, kT)):
                    for m in range(2):
                        for j in range(SB):
                            ps = bps.tile([128, 512], F32, tag="mm", name="mm")
                            for e in range(ECH):
                                nc.tensor.matmul(
                                    ps[:],
                                    w_t_[:, e, m * 128:(m + 1) * 128],
                                    xt[e][:, ts(j, 512)],
                                    start=(e == 0),
                                    stop=(e == ECH - 1),
                                )
                            # alternate psum evictions DVE/ACT (ACT is
                            # otherwise idle during projections)
                            if nev % 2 == 0:
                                nc.vector.tensor_copy(
                                    out=dest[m][:, ts(j, 512)], in_=ps[:]
                                )
                            else:
                                nc.scalar.copy(
                                    out=dest[m][:, ts(j, 512)], in_=ps[:]
                                )
                            nev += 1
                        if m == 0:
                            rope(dest, m)
                # v projection -> seq-partition tiles with ones column
                for t in range(SCH):
                    nc.gpsimd.memset(
                        vt[t].rearrange("p (h c) -> p h c", c=D + 1)[:, :, D:D + 1],
                        1.0,
                    )
                    ps = bps.tile([128, HL], F32, tag="mm", name="mmv")
                    for e in range(ECH):
                        nc.tensor.matmul(
                            ps[:],
                            xt[e][:, ts(t, 128)],
                            wv_t[:, e, :],
                            start=(e == 0),
                            stop=(e == ECH - 1),
                        )
                    if t % 2 == 0:
                        nc.vector.tensor_copy(
                            out=vt[t].rearrange("p (h c) -> p h c", c=D + 1)[:, :, 0:D],
                            in_=ps.rearrange("p (h c) -> p h c", c=D),
                        )
                    else:
                        nc.scalar.copy(
                            out=vt[t].rearrange("p (h c) -> p h c", c=D + 1)[:, :, 0:D],
                            in_=ps.rearrange("p (h c) -> p h c", c=D),
                        )
                rope(qT, 1)
                rope(kT, 1)

            # ---------------- Phase C: attention, two heads in lockstep -------
            with (
                tc.tile_pool(name="cexp", bufs=6) as cexp,
                tc.tile_pool(name="cz", bufs=4) as cz,
                tc.tile_pool(name="crb", bufs=3) as crb,
                tc.tile_pool(name="csc", bufs=3, space="PSUM") as csc,
                tc.tile_pool(name="cpv", bufs=1, space="PSUM") as cpv,
            ):
                def qksv(c):
                    hs = [2 * c, 2 * c + 1]
                    qs = [qT[c][0:64, :], qT[c][64:128, :]]
                    ks = [kT[c][0:64, :], kT[c][64:128, :]]
                    vs = [
                        [vt[t].rearrange("p (h c) -> p h c", c=D + 1)[:, h, :]
                         for t in range(SCH)]
                        for h in hs
                    ]
                    return hs, qs, ks, vs

                # software-pipelined per (block, slab, chunk): per-chunk score
                # tiles hold both heads side by side, blocks interleave slabs
                units = []
                for j in range(SB):
                    nt = 4 * (j + 1)
                    for c in range(2):
                        for t in range(nt):
                            units.append((c, j, t, nt))
                sc_of = {}
                pv_of = {}

                def trim_off(t, nt):
                    # the two trailing diagonal chunks are mostly masked:
                    # width-trim their score/exp/pv path
                    if t == nt - 2:
                        return 256
                    if t == nt - 1:
                        return 384
                    return 0

                def emit_sc(u):
                    c, j, t, nt = u
                    _, qs, ks, _ = qksv(c)
                    off = trim_off(t, nt)
                    sc = csc.tile([128, 1024], F32, tag="sc", name="sc")
                    # head 0 on PE rows 0-63, head 1 on rows 64-127: the two
                    # matmuls overlap in the array
                    for i in range(2):
                        nc.tensor.matmul(
                            sc[:, i * 512 + off:(i + 1) * 512],
                            ks[i][:, ts(t, 128)],
                            qs[i][:, j * 512 + off:(j + 1) * 512],
                            start=True,
                            stop=True,
                        )
                    sc_of[u] = sc

                def emit_consume(u):
                    c, j, t, nt = u
                    hs, _, _, vs = qksv(c)
                    if t == 0:
                        pv_of[(c, j)] = [
                            cpv.tile([65, 512], F32, tag=f"pv{i}", name=f"pv{i}")
                            for i in range(2)
                        ]
                    pv = pv_of[(c, j)]
                    sc = sc_of.pop(u)
                    off = trim_off(t, nt)
                    exm = cexp.tile([128, 1024], F16, tag="ex", name="ex")
                    if off:
                        nc.scalar.activation(
                            out=exm.rearrange("p (h q) -> p h q", h=2)[:, :, off:512],
                            in_=sc.rearrange("p (h q) -> p h q", h=2)[:, :, off:512],
                            func=Exp, scale=0.125,
                        )
                    else:
                        nc.scalar.activation(
                            out=exm[:], in_=sc[:], func=Exp, scale=0.125
                        )
                    if t >= nt - 4:  # diagonal chunk: causal mask, both heads
                        nc.gpsimd.affine_select(
                            out=exm.rearrange("p (h q) -> p h q", h=2)[:, :, off:512],
                            in_=exm.rearrange("p (h q) -> p h q", h=2)[:, :, off:512],
                            compare_op=is_ge,
                            fill=0.0,
                            base=(j * 512 - t * 128) + off,
                            channel_multiplier=-1,
                            pattern=[[0, 2], [1, 512 - off]],
                        )
                    for i in range(2):
                        nc.tensor.matmul(
                            pv[i][:, off:512],
                            vs[i][t],
                            exm[:, i * 512 + off:(i + 1) * 512],
                            start=(t == 0),
                            stop=(t == nt - 1),
                        )
                    if t == nt - 1:
                        # end of block: evict + per-block softmax normalization
                        # (Z -> DRAM -> partition-broadcast, invert, scale)
                        zq = cz.tile([65, 2, 512], F32, tag="zq", name="zq")
                        for i in range(2):
                            nc.vector.tensor_copy(
                                out=oT[c][i * 64:(i + 1) * 64, ts(j, 512)],
                                in_=pv[i][0:64, :],
                            )
                            nc.vector.tensor_copy(
                                out=zq[64:65, i, :], in_=pv[i][64:65, :]
                            )
                        for i in range(2):
                            nc.sync.dma_start(
                                out=zscr_d[hs[i], ts(j, 512)],
                                in_=zq[64:65, i, :],
                            )
                        rb = crb.tile([128, 512], F32, tag="rb", name="rb")
                        for i in range(2):
                            nc.sync.dma_start(
                                out=rb[i * 64:(i + 1) * 64, :],
                                in_=zscr_d[hs[i]:hs[i] + 1, ts(j, 512)]
                                .to_broadcast((64, 512)),
                            )
                        rbr = crb.tile([128, 512], F32, tag="rbr", name="rbr")
                        nc.vector.reciprocal_approx_fast(out=rbr[:], in_=rb[:])
                        nc.vector.tensor_mul(
                            out=oT[c][:, ts(j, 512)],
                            in0=oT[c][:, ts(j, 512)],
                            in1=rbr[:],
                        )

                emit_sc(units[0])
                for un in range(1, len(units)):
                    emit_sc(units[un])
                    emit_consume(units[un - 1])
                emit_consume(units[-1])

            # ---------------- Phase D: output projection (row-split Wo) --------
            with tc.tile_pool(name="dps", bufs=8, space="PSUM") as dps:
                oq = [nc.sync, nc.gpsimd]
                for t in range(SCH):
                    for n in range(2):
                        ps = dps.tile([128, 512], F32, tag="wo", name="wops")
                        for c in range(2):
                            nc.tensor.matmul(
                                ps[:],
                                oT[c][:, ts(t, 128)],
                                wo_t[:, c, ts(n, 512)],
                                start=(c == 0),
                                stop=(c == 1),
                            )
                        ot = ev.tile([128, 512], F16, tag="out", name="oev")
                        if (2 * t + n) % 2 == 0:
                            nc.vector.tensor_copy(out=ot[:], in_=ps[:])
                        else:
                            nc.scalar.copy(out=ot[:], in_=ps[:])
                        oq[(2 * t + n) % 2].dma_start(
                            out=out_d[ts(t, 128), ts(n, 512)], in_=ot[:]
                        )

    nc.compile()
    return nc


def _rope_tables():
    iexp = np.arange(0, D, 2, dtype=np.float32) / np.float32(D)
    inv_freq = np.reciprocal(np.power(np.float32(ROPE_BASE), iexp))  # (32,) f32
    ang = np.arange(S, dtype=np.float32)[:, None] * inv_freq[None, :]  # (S, 32)
    cos = np.cos(ang).astype(np.float32)  # (S, 32)
    sin = np.sin(ang).astype(np.float32)
    cosx = np.empty((64, S), dtype=np.float32)
    sinx = np.empty((64, S), dtype=np.float32)
    cosx[0::2] = cos.T
    cosx[1::2] = cos.T
    sinx[0::2] = -sin.T
    sinx[1::2] = sin.T
    return (np.tile(cosx, (2, 1)).astype(np.float16),
            np.tile(sinx, (2, 1)).astype(np.float16))  # (128, S) each


def get_nc():
    global _built
    if _built is None:
        _built = _build_nc()
    return _built


def make_in_maps(x, Wq, Wk, Wv, Wo):
    cosx, sinx = _rope_tables()
    in_maps = []
    for c in range(NCORES):
        b, g = c // 4, c % 4
        sl = slice(g * HL, (g + 1) * HL)
        in_maps.append({
            "xT": np.ascontiguousarray(x[b].T).astype(np.float16),
            "wq": np.ascontiguousarray(Wq[:, sl]).astype(np.float16),
            "wk": np.ascontiguousarray(Wk[:, sl]).astype(np.float16),
            "wv": np.ascontiguousarray(Wv[:, sl]).astype(np.float16),
            "wo": np.ascontiguousarray(Wo[sl, :]).astype(np.float16),
            "cosx": cosx,
            "sinx": sinx,
        })
    return in_maps


def gather(results):
    out = np.empty((B, S, E), dtype=np.float32)
    for b in range(B):
        acc = results[4 * b]["out"].astype(np.float32)
        for g in range(1, 4):
            acc += results[4 * b + g]["out"].astype(np.float32)
        out[b] = acc
    return out


def kernel(x, Wq, Wk, Wv, Wo):
    from concourse.bass_utils import run_bass_kernel_spmd

    nc = get_nc()
    in_maps = make_in_maps(
        np.asarray(x), np.asarray(Wq), np.asarray(Wk), np.asarray(Wv), np.asarray(Wo)
    )
    res = run_bass_kernel_spmd(nc, in_maps, list(range(NCORES)))
    return gather(res.results)
